# revision 48
# baseline (speedup 1.0000x reference)
"""Trainium2 Bass kernel: 16-head causal self-attention block (QKV proj ->
causal MHA -> output proj) on 8 NeuronCores.

Contract: kernel(**inputs) takes FULL unsharded inputs
  x      [2, 2048, 1024] f32
  w_qkv  [1024, 3072] f32, b_qkv [3072] f32
  w_proj [1024, 1024] f32, b_proj [1024] f32
and returns the FULL output [2, 2048, 1024] f32.

Sharding: (batch x head-group). Core c owns batch c//4 and heads
4*(c%4) .. 4*(c%4)+3:
  - column-parallel QKV (256 q/k/v feature columns per core)
  - full causal attention for its 4 heads over its batch's 2048 tokens
  - row-parallel output projection -> partial [2048, 1024] f16 sums
  - host reduces 4 partials per batch and adds b_proj.

All matmul operands are f16 (PSUM accumulation stays f32); rel-err vs the
f32 reference lands ~1e-3, far inside the 2e-2 gate.

Per-core dataflow, engineered around three hardware facts:
  1. PE HAM clock gate: the PE runs 2.4 GHz only while continuously busy,
     else 1.2 GHz -> the emission order interleaves QKV-for-next-q-tile and
     projection-for-previous-q-tile between attention blocks so the greedy
     Tile scheduler always has ready PE work during exp waits.
  2. ACT activation-table thrash (Exp vs Ln): softmax denominators are
     inverted with nc.vector.reciprocal + a K=2 broadcast matmul, so the
     ACT engine only ever uses the Exp/Identity table.
  3. Two HWDGE FIFOs: weight loads + x transposes split across the
     sync/scalar FIFOs; output writes go on the scalar FIFO after its
     transposes are done (one xbar-mode transition per FIFO).

Layouts: x is DMA-transposed to xT [E-chunk, tok]. q,k are produced
feature-major (qT/kT [128, 512] per pair-of-heads per 512-token super)
for the S^T = K^T.T Q matmuls; v is produced in natural token-major
layout directly by swapping matmul operands (lhsT=xT chunk, rhs=wv),
which is exactly the [k-token, feat] layout the AV matmul needs -- no PE
transposes at all. The AV lhsT is v augmented with a ones column per head
(M=65) so PSUM row 64 accumulates the softmax denominator Z for free.
"""

import numpy as np
from contextlib import ExitStack

import concourse.bass as bass
import concourse.tile as tile
from concourse import bacc, mybir
from concourse.bass_utils import run_bass_kernel_spmd
from concourse.masks import make_upper_triangular

F32 = mybir.dt.float32
F32R = mybir.dt.float32r
F16 = mybir.dt.float16
AF = mybir.ActivationFunctionType

N_CORES = 8
N_WARM = 0
B, T, E, H, D = 2, 2048, 1024, 16, 64
P = 128
HPC = 4              # heads per core
FPC = HPC * D        # 256 feature columns per core for each of q,k,v
SUPER = 512          # tokens per QKV super-tile / attention q-tile
NS = T // SUPER      # 4 super-tiles (== q-tiles)
KCH = E // P         # 8 contraction chunks of x
NT = T // P          # 16 token tiles
VAW = HPC * (D + 1)  # 260 va columns per token tile


def _emit(nc, tc, ctx, with_vbias):
    # all tensors arrive in their exact SBUF layouts (host pre-transposes),
    # so every DMA is a plain copy -- no DMA transposes, no xbar-mode
    # serialization between transpose and copy traffic.
    x_h = nc.declare_dram_parameter("x", [E, T], F16, isOutput=False)
    wq_h = nc.declare_dram_parameter("wq", [P, KCH * FPC], F16, isOutput=False)
    wk_h = nc.declare_dram_parameter("wk", [P, KCH * FPC], F16, isOutput=False)
    wv_h = nc.declare_dram_parameter("wv", [P, KCH * FPC], F16, isOutput=False)
    bq_h = nc.declare_dram_parameter("bq", [P, 2], F32, isOutput=False)
    bk_h = nc.declare_dram_parameter("bk", [P, 2], F32, isOutput=False)
    bv_h = nc.declare_dram_parameter("bv", [1, FPC], F16, isOutput=False)
    wp_h = nc.declare_dram_parameter("wp", [P, 2 * E], F16, isOutput=False)
    out_h = nc.declare_dram_parameter("out", [T, E], F16, isOutput=True)

    outr = out_h[:].rearrange("(n p) e -> n p e", p=P)  # [16, 128, 1024]

    const = ctx.enter_context(tc.tile_pool(name="const", bufs=1))
    persist = ctx.enter_context(tc.tile_pool(name="persist", bufs=1))

    # ---- constants ----
    mask_tri = const.tile([P, P], F16)  # mask[p, f] = 1.0 iff p <= f
    make_upper_triangular(nc, mask_tri[:], val=1.0, diag=True)
    ones_r = const.tile([1, P], F16)    # rank-1 lhsT for the v bias
    nc.vector.memset(ones_r[:], 1.0)
    # Z-broadcast selector: ys_h holds y rows 0:63 and Z at row 64.
    # matmul(lhsT=e65, rhs=ys_h) copies row 64 into all 64 out partitions.
    e65f = const.tile([65, D], F32)
    nc.vector.memset(e65f[:], 0.0)
    nc.vector.memset(e65f[64:65, :], 1.0)
    e65 = const.tile([65, D], F16)
    nc.vector.tensor_copy(e65[:], e65f[:])

    # ---- weights + xT to SBUF, ordered so QKV(0) can start ~3us in ----
    wq_sb = const.tile([P, KCH * FPC], F16)   # [128, 2048]: chunk ch at cols ch*256
    wk_sb = const.tile([P, KCH * FPC], F16)
    wv_sb = const.tile([P, KCH * FPC], F16)
    wp_sb = const.tile([P, 2 * E], F16)       # [128, 2048]: pair pr at cols pr*1024
    bq_sb = const.tile([P, 2], F32)
    bk_sb = const.tile([P, 2], F32)
    bv_sb = const.tile([1, FPC], F16)
    xT = [persist.tile([P, T], F16, tag=f"xT{ch}", name=f"xT{ch}")
          for ch in range(KCH)]

    # all loads on the sync FIFO, keeping the ACT queue free for exp
    nc.sync.dma_start(wq_sb[:], wq_h[:])
    for ch in range(KCH):   # super-0 tokens first
        nc.sync.dma_start(xT[ch][:, 0:SUPER], x_h[ch * P:(ch + 1) * P, 0:SUPER])
    nc.sync.dma_start(bq_sb[:], bq_h[:])
    nc.sync.dma_start(wk_sb[:], wk_h[:])
    nc.sync.dma_start(bk_sb[:], bk_h[:])
    nc.sync.dma_start(wv_sb[:], wv_h[:])
    nc.sync.dma_start(bv_sb[:], bv_h[:])
    nc.sync.dma_start(wp_sb[:], wp_h[:])
    for ch in range(KCH):
        nc.sync.dma_start(xT[ch][:, SUPER:T], x_h[ch * P:(ch + 1) * P, SUPER:T])

    # ---- persistent activations ----
    qT = [[persist.tile([P, SUPER], F16, tag=f"qT{pr}_{s}", name=f"qT{pr}_{s}")
           for s in range(NS)] for pr in range(2)]
    kT = [[persist.tile([P, SUPER], F16, tag=f"kT{pr}_{s}", name=f"kT{pr}_{s}")
           for s in range(NS)] for pr in range(2)]
    va = [persist.tile([P, VAW], F16, tag=f"va{ti}", name=f"va{ti}")
          for ti in range(NT)]
    for ti in range(NT):   # preset the ones columns (col 64 of each head's 65)
        nc.vector.memset(
            va[ti][:].rearrange("p (h x) -> p h x", h=HPC)[:, :, D:D + 1], 1.0)

    poolQ = ctx.enter_context(tc.tile_pool(name="poolQ", bufs=2, space="PSUM"))
    poolS = ctx.enter_context(tc.tile_pool(name="poolS", bufs=2, space="PSUM"))
    poolY = ctx.enter_context(tc.tile_pool(name="poolY", bufs=2, space="PSUM"))
    ptpool = ctx.enter_context(tc.tile_pool(name="ptp", bufs=4))
    ytpool = ctx.enter_context(tc.tile_pool(name="ytp", bufs=4))
    zpool = ctx.enter_context(tc.tile_pool(name="zp", bufs=2))
    opool = ctx.enter_context(tc.tile_pool(name="op", bufs=3))

    # PE warm-up: ~3.5us of throwaway matmuls while the first DMAs land, so
    # the HAM clock-gate opens before real work starts (idle PE = 1.2 GHz,
    # and the un-throttle needs a full busy window)
    warm = const.tile([1, SUPER], F16)
    nc.vector.memset(warm[:], 0.0)
    pwarm = poolQ.tile([P, SUPER], F32, tag="pq", name="pwarm")
    for _ in range(N_WARM):
        nc.tensor.matmul(pwarm[:], lhsT=warm[0:1, 0:P], rhs=warm[:],
                         start=True, stop=True)

    def emit_qkv(s, w0=None, dw=0.0):
      """Emit QKV for super-tile s. If w0 is given, hold successive chains
      until sim-times w0, w0+dw, ... (ms) so the scheduler spreads them as
      PE filler across the concurrently-running attention phase instead of
      burning them all during the first exp stall."""
      with nc.named_scope(f"qkv{s}"):
        t0 = s * SUPER
        ci = 0

        def hold():
            nonlocal ci
            ci += 1
            if w0 is None:
                return tc.tile_wait_until(0.0, enable=False)
            return tc.tile_wait_until(w0[ci - 1], enable=True)

        # q then k: feature-major, 2 pairs (M-chunks of 128)
        for wsb, bsb, dst in ((wq_sb, bq_sb, qT), (wk_sb, bk_sb, kT)):
            for pr in range(2):
              with hold():
                pf = poolQ.tile([P, SUPER], F32, tag="pq", name="pf")
                for ch in range(KCH):
                    nc.tensor.matmul(
                        pf[:],
                        lhsT=wsb[:, ch * FPC + pr * P: ch * FPC + (pr + 1) * P],
                        rhs=xT[ch][:, t0:t0 + SUPER],
                        start=(ch == 0), stop=(ch == KCH - 1))
                nc.vector.tensor_scalar_add(dst[pr][s][:], pf[:],
                                            bsb[:, pr:pr + 1])
        # v: natural token-major, one chain per 128-token tile
        for tt4 in range(4):
            ti = s * 4 + tt4
            with hold():
                pv = poolQ.tile([P, FPC], F32, tag="pq", name="pv")
                if with_vbias:
                    nc.tensor.matmul(pv[:], lhsT=ones_r[0:1, :],
                                     rhs=bv_sb[0:1, :], start=True, stop=False)
                for ch in range(KCH):
                    nc.tensor.matmul(
                        pv[:],
                        lhsT=xT[ch][:, ti * P:(ti + 1) * P],
                        rhs=wv_sb[:, ch * FPC:(ch + 1) * FPC],
                        start=(ch == 0 and not with_vbias),
                        stop=(ch == KCH - 1))
                nc.vector.tensor_copy(
                    va[ti][:].rearrange("p (h x) -> p h x", h=HPC)[:, :, 0:D],
                    pv[:].rearrange("p (h x) -> p h x", h=HPC))

    # Deferred PE work (norm part-B / proj pieces of earlier q-tiles) popped
    # one piece after each attention block: the PE queue then never
    # head-of-line blocks on a matmul whose DVE inputs are still in flight,
    # and the boundary stalls that re-arm the HAM throttle disappear.
    fillers = []

    def emit_attn(qi):
        nkb = 4 * (qi + 1)
        yts = [None, None]
        for pr in range(2):
          with nc.named_scope(f"at{qi}_{pr}"):
            pys = [poolY.tile([D + 1, SUPER], F32, tag="y", name=f"py{h}")
                   for h in range(2)]
            for kb in range(nkb):
                c0 = max(0, kb * P - qi * SUPER)
                diag = kb * P >= qi * SUPER
                sk, kc = kb // 4, (kb % 4) * P
                ps = poolS.tile([P, 2 * SUPER], F32, tag="s", name="ps")
                for h in range(2):
                    nc.tensor.matmul(
                        ps[:, h * SUPER + c0:(h + 1) * SUPER],
                        lhsT=kT[pr][sk][h * D:(h + 1) * D, kc:kc + P],
                        rhs=qT[pr][qi][h * D:(h + 1) * D, c0:SUPER],
                        start=True, stop=True)
                pt = ptpool.tile([P, 2 * SUPER], F16, tag="pt", name="pt")
                if c0 == 0:
                    nc.scalar.activation(pt[:], ps[:], AF.Exp, scale=0.125)
                else:
                    src = ps[:].rearrange("p (h q) -> p h q", h=2)[:, :, c0:]
                    dst = pt[:].rearrange("p (h q) -> p h q", h=2)[:, :, c0:]
                    nc.scalar.activation(dst, src, AF.Exp, scale=0.125)
                if diag:
                    sl = pt[:].rearrange("p (h q) -> p h q",
                                         h=2)[:, :, c0:c0 + P]
                    m3 = mask_tri[:].rearrange(
                        "p (u f) -> p u f", u=1).broadcast_to([P, 2, P])
                    nc.vector.tensor_mul(sl, sl, m3)
                for h in range(2):
                    hh = 2 * pr + h
                    nc.tensor.matmul(
                        pys[h][0:D + 1, c0:SUPER],
                        lhsT=va[4 * sk + kb % 4][:, hh * (D + 1):
                                                 (hh + 1) * (D + 1)],
                        rhs=pt[:, h * SUPER + c0:(h + 1) * SUPER],
                        start=(kb == 0), stop=(kb == nkb - 1))
                if fillers:
                    fillers.pop(0)()
            # normalize part A (DVE only, right away): evacuate y+Z together,
            # releasing the pys PSUM banks for the next pair's AV chain
            with nc.named_scope(f"nm{qi}_{pr}"):
                ys = [zpool.tile([D + 1, SUPER], F16, tag="ys", name=f"ys{h}")
                      for h in range(2)]
                for h in range(2):
                    nc.vector.tensor_copy(ys[h][:], pys[h][0:D + 1, :])

            def norm_b(qi=qi, pr=pr, ys=ys):
              # part B: broadcast Z via rank-1 matmuls, invert, scale
              with nc.named_scope(f"nb{qi}_{pr}"):
                pz = poolS.tile([D, 2 * SUPER], F32, tag="s", name="pz")
                for h in range(2):
                    nc.tensor.matmul(pz[:, h * SUPER:(h + 1) * SUPER],
                                     lhsT=e65[:], rhs=ys[h][:],
                                     start=True, stop=True)
                zb = zpool.tile([D, 2 * SUPER], F32, tag="zb", name="zb")
                nc.vector.reciprocal_approx_fast(zb[:], pz[:])
                yt = ytpool.tile([P, SUPER], F16, tag="yt", name="yt")
                nc.vector.tensor_mul(yt[0:D, :], ys[0][0:D, :],
                                     zb[:, 0:SUPER])
                nc.vector.tensor_mul(yt[D:P, :], ys[1][0:D, :],
                                     zb[:, SUPER:2 * SUPER])
                yts[pr] = yt

            fillers.append(norm_b)

        def proj_piece(tt4, qi=qi, yts=yts):
          with nc.named_scope(f"pj{qi}_{tt4}"):
            ti = qi * 4 + tt4
            ot = opool.tile([P, E], F16, tag="ot", name="ot")
            for oc in range(2):
                pp = poolQ.tile([P, SUPER], F32, tag="pq", name="pp")
                for pr in range(2):
                    nc.tensor.matmul(
                        pp[:],
                        lhsT=yts[pr][:, tt4 * P:(tt4 + 1) * P],
                        rhs=wp_sb[:, pr * E + oc * SUPER:
                                  pr * E + (oc + 1) * SUPER],
                        start=(pr == 0), stop=(pr == 1))
                nc.vector.tensor_copy(ot[:, oc * SUPER:(oc + 1) * SUPER],
                                      pp[:])
            nc.sync.dma_start(outr[ti], ot[:])

        for tt4 in range(4):
            fillers.append(lambda tt4=tt4: proj_piece(tt4))

    # Emission order = scheduler priority. Attention (and its norm/proj) is
    # emitted first so it always wins ties; the remaining QKV supers go last,
    # making their chains pure PE gap-filler: they run exactly when the
    # attention pipeline stalls on exp, which both hides the stalls and keeps
    # the PE duty high enough that the HAM clock-gate stays at 2.4 GHz.
    # Dataflow deps (attn(s) reads qT/kT/va of supers <= s) keep it correct.
    W2 = [0.020, 0.024, 0.028, 0.032, 0.036, 0.040, 0.044, 0.048]
    W3 = [0.050, 0.055, 0.060, 0.070, 0.080, 0.090, 0.100, 0.110]
    emit_qkv(0)
    emit_qkv(1)
    for qi in range(NS):
        emit_attn(qi)
        if qi + 2 < NS:
            emit_qkv(qi + 2, w0=(W2, W3)[qi])
    for f in fillers:
        f()
    fillers.clear()


_NC_CACHE = {}


def _build(with_vbias=False):
    nc = _NC_CACHE.get(with_vbias)
    if nc is None:
        nc = bacc.Bacc("TRN2", target_bir_lowering=False, debug=False)
        with tile.TileContext(nc) as tc:
            with ExitStack() as ctx:
                _emit(nc, tc, ctx, with_vbias)
        nc.compile()
        _NC_CACHE[with_vbias] = nc
    return nc


def _chunked(w):
    """[E, M] weight slice -> [128, KCH*M] f16: chunk ch of 128 E-rows lands
    at columns ch*M, matching the lhsT/rhs SBUF layout."""
    e, m = w.shape
    return np.ascontiguousarray(
        w.reshape(e // P, P, m).transpose(1, 0, 2).reshape(P, -1)
        .astype(np.float16))


def make_in_maps(x, w_qkv, b_qkv, w_proj):
    x = np.asarray(x, dtype=np.float32)
    w_qkv = np.asarray(w_qkv, dtype=np.float32)
    b_qkv = np.asarray(b_qkv, dtype=np.float32)
    w_proj = np.asarray(w_proj, dtype=np.float32)
    xb = [np.ascontiguousarray(x[b].T.astype(np.float16)) for b in range(B)]
    in_maps = []
    for c in range(N_CORES):
        b, hg = c // 4, c % 4
        lo = FPC * hg
        in_maps.append({
            "x": xb[b],
            "wq": _chunked(w_qkv[:, lo:lo + FPC]),
            "wk": _chunked(w_qkv[:, E + lo:E + lo + FPC]),
            "wv": _chunked(w_qkv[:, 2 * E + lo:2 * E + lo + FPC]),
            "bq": np.ascontiguousarray(
                b_qkv[lo:lo + FPC].reshape(2, P).T.astype(np.float32)),
            "bk": np.ascontiguousarray(
                b_qkv[E + lo:E + lo + FPC].reshape(2, P).T.astype(np.float32)),
            "bv": np.ascontiguousarray(
                b_qkv[2 * E + lo:2 * E + lo + FPC]
                .reshape(1, FPC).astype(np.float16)),
            "wp": _chunked(w_proj[lo:lo + FPC, :]),
        })
    return in_maps


def run_sharded(inputs, trace=False, **kw):
    nc = _build(with_vbias=bool(
        np.any(np.asarray(inputs["b_qkv"], dtype=np.float32)[2 * E:])))
    in_maps = make_in_maps(inputs["x"], inputs["w_qkv"], inputs["b_qkv"],
                           inputs["w_proj"])
    res = run_bass_kernel_spmd(nc, in_maps, list(range(N_CORES)),
                               trace=trace, **kw)
    bp = np.asarray(inputs["b_proj"], dtype=np.float32)
    out = np.zeros((B, T, E), dtype=np.float32)
    for c in range(N_CORES):
        out[c // 4] += res.results[c]["out"].astype(np.float32)
    out += bp[None, None, :]
    return out, res


def kernel(**inputs) -> np.ndarray:
    out, _ = run_sharded(inputs, trace=False)
    return out
